# revision 8
# baseline (speedup 1.0000x reference)
"""Self-contained Trainium2 Bass kernel for the 4-layer GraphSAGE GNN
(nn_EnhancedClassifier): kernel(**inputs) -> np.ndarray [100000] f32.

Runs SPMD on 8 NeuronCores via run_bass_kernel_spmd.

Strategy: dst-partition nodes across 8 cores (12500 each). Host sorts
edges by (dst_core, dst_block, src_chunk), pads each (block, chunk)
edge group to a per-group tile count (max over cores, so one SPMD
program works for all 8). On device, per layer per block:
  indirect-DMA gather of src rows (bf16, 256B) -> one-hot build (DVE)
  -> TensorE scatter-accumulate giving aggT [feat, dst] -> dense
  matmuls; deg_inv scaling + LayerNorm run on the Scalar engine to
  keep DVE free for the one-hot builds. h is bf16 node-major in DRAM;
  an AllGather shares it between layers.
"""
import sys
sys.path.insert(0, '/opt/trn_rl_repo')
import numpy as np
import ml_dtypes
from concourse import bass, bacc, mybir, tile

BF16 = mybir.dt.bfloat16
F32 = mybir.dt.float32
AF = mybir.ActivationFunctionType
ALU = mybir.AluOpType

NCORES = 8

# --- Patch Tile's DMASW lane assignment to be SWDGE-queue-aware: lane%4 must
# equal the instruction's queue_num or the runtime rejects the sem update.
import concourse.tile_sem_assignment as _tsa
from concourse import bass_isa as _bisa

if not getattr(_tsa, "_gnn_queue_patch", False):
    _orig_assign_tick = _tsa.TileClockTick._assign_tick

    def _assign_tick_qaware(self, inst):
        if isinstance(inst, mybir.InstDMAGatherAnt):
            q = inst.queue_num
            rot = self.__dict__.setdefault("_gnn_qrot", {})
            k = rot.get(q, 0)
            rot[q] = k ^ 1
            self.next_sw_dma_idx = q + 4 * k
        elif (isinstance(inst, _tsa.DMAInst)
              and inst.engine == mybir.EngineType.Pool
              and not isinstance(inst, _bisa.UserSyncedRemoteDMADescs)):
            rot = self.__dict__.setdefault("_gnn_qrot", {})
            k = rot.get(0, 0)
            rot[0] = k ^ 1
            self.next_sw_dma_idx = 4 * k
        return _orig_assign_tick(self, inst)

    _tsa.TileClockTick._assign_tick = _assign_tick_qaware
    _tsa._gnn_queue_patch = True

IN_F = 64
GF = 128            # gather row width (bf16 -> 256B rows, all layers)
HID = 128
PAD_DSTLOC = 1000.0
NCHUNK = 4
WT = 8              # gather tiles (of 128 rows) per dma_gather call


class Cfg:
    def __init__(self, n_nodes):
        self.N = n_nodes
        self.NPC = n_nodes // NCORES
        assert self.NPC * NCORES == self.N
        self.B = (self.NPC + 127) // 128
        self.ROWS = self.B * 128
        self.GROWS = self.ROWS * NCORES
        assert self.GROWS % NCHUNK == 0
        self.CHUNK = self.GROWS // NCHUNK
        assert self.CHUNK <= 32768


def preprocess(cfg, x, edge_index, weights):
    """Host-side: partition + sort edges, build per-core gather metadata.

    Slot layout (same for every core): chunk streams concatenated; within
    stream ch, blocks in order, block b owning tiles_bc[b][ch] tiles of 128
    slots. tiles_bc is the max over cores so one program fits all."""
    B = cfg.B
    src = edge_index[0].astype(np.int64)
    dst = edge_index[1].astype(np.int64)

    deg = np.bincount(dst, minlength=cfg.N).astype(np.float32)
    deginv = 1.0 / np.maximum(deg, 1.0)

    core_of = src // cfg.NPC
    pad_row_src = (core_of * cfg.ROWS + src % cfg.NPC).astype(np.int64)
    src_chunk = pad_row_src // cfg.CHUNK

    dst_core = dst // cfg.NPC
    dst_local = (dst % cfg.NPC).astype(np.int64)
    dst_block = dst_local // 128

    cnt = np.zeros((NCORES, B, NCHUNK), np.int64)
    np.add.at(cnt, (dst_core, dst_block, src_chunk), 1)
    tiles_bc = np.ceil(cnt.max(axis=0) / 128).astype(np.int64)   # [B, NCHUNK]

    # stream/tile bookkeeping (shared across cores)
    CT = tiles_bc.sum(axis=0)                       # tiles per chunk stream
    tile_start = np.zeros((B, NCHUNK), np.int64)    # block's first tile in stream
    tile_start[1:] = np.cumsum(tiles_bc, axis=0)[:-1]
    stream_base = np.zeros(NCHUNK, np.int64)
    stream_base[1:] = np.cumsum(CT)[:-1]
    T_b = tiles_bc.sum(axis=1)                      # one-hot tiles per block
    dcol_start = np.zeros(B, np.int64)
    dcol_start[1:] = np.cumsum(T_b)[:-1]
    boff = np.zeros((B, NCHUNK), np.int64)          # chunk offset inside block
    boff[:, 1:] = np.cumsum(tiles_bc, axis=1)[:, :-1]
    TOT_TILES = int(T_b.sum())
    TOT_SLOTS = TOT_TILES * 128
    cfg.tiles_bc, cfg.tile_start, cfg.stream_base = tiles_bc, tile_start, stream_base
    cfg.CT, cfg.T_b, cfg.dcol_start = CT, T_b, dcol_start
    cfg.TOT_TILES = TOT_TILES
    cfg.Tmax = int(T_b.max())

    # bf16 x padded to GF features
    x_pad = np.zeros((cfg.GROWS, GF), ml_dtypes.bfloat16)
    for c in range(NCORES):
        x_pad[c * cfg.ROWS:c * cfg.ROWS + cfg.NPC, :IN_F] = (
            x[c * cfg.NPC:(c + 1) * cfg.NPC].astype(ml_dtypes.bfloat16))

    order = np.lexsort((dst_local, src_chunk, dst_block, dst_core))
    s_src_row = pad_row_src[order]
    s_dst_loc = dst_local[order]
    key = (dst_core[order] * B + dst_block[order]) * NCHUNK + src_chunk[order]
    group_starts = np.searchsorted(key, np.arange(NCORES * B * NCHUNK + 1))

    W = {k: np.asarray(v) for k, v in weights.items()}
    wcast = {}
    for k in ["Wl1", "Wr1", "Wres", "Wl2", "Wr2", "Wl3", "Wr3", "Wl4", "Wr4"]:
        wcast[k] = W[k].astype(ml_dtypes.bfloat16)
    brow = {}
    for k in ["b1", "bres", "b2", "b3"]:
        brow[k] = W[k].reshape(1, HID).astype(ml_dtypes.bfloat16)
    brow["b4"] = W["b4"].reshape(1, 1).astype(ml_dtypes.bfloat16)
    ln_g = W["ln_g"].astype(np.float32)
    ln_b = W["ln_b"].astype(np.float32)
    ln_identity = bool(np.all(ln_g == 1.0) and np.all(ln_b == 0.0))
    ln_g_rep = np.broadcast_to(ln_g, (128, HID)).copy()
    ln_b_rep = np.broadcast_to(ln_b, (128, HID)).copy()

    iota_bf = np.broadcast_to(np.arange(128, dtype=np.float32), (128, 128)).astype(
        ml_dtypes.bfloat16).copy()
    ones_row = np.ones((1, 128), ml_dtypes.bfloat16)
    ident_bf = np.eye(128, dtype=ml_dtypes.bfloat16)

    in_maps = []
    for c in range(NCORES):
        idx_lin = np.zeros(TOT_SLOTS, np.int16)
        dstloc = np.full((128, TOT_TILES), PAD_DSTLOC, np.float32)
        for b in range(B):
            for ch in range(NCHUNK):
                g = (c * B + b) * NCHUNK + ch
                lo, hi = group_starts[g], group_starts[g + 1]
                n = hi - lo
                if n == 0:
                    continue
                base = (stream_base[ch] + tile_start[b, ch]) * 128
                sidx = np.arange(n)
                idx_lin[base + sidx] = (s_src_row[lo:hi] - ch * cfg.CHUNK
                                        ).astype(np.int16)
                t_i = sidx // 128
                p_i = sidx % 128
                dstloc[p_i, dcol_start[b] + boff[b, ch] + t_i] = (
                    s_dst_loc[lo:hi] - b * 128).astype(np.float32)
        idx_pk = idx_lin.reshape(TOT_SLOTS // 16, 16).T     # [16, COLS16]
        idx_pk = np.tile(idx_pk, (8, 1))                    # 128 partitions

        dinv_col = np.ones((128, B), np.float32)
        base = c * cfg.NPC
        for b in range(B):
            n_real = min(128, cfg.NPC - b * 128)
            dinv_col[:n_real, b] = deginv[base + b * 128: base + b * 128 + n_real]
        x_own = np.ascontiguousarray(
            x_pad[c * cfg.ROWS:(c + 1) * cfg.ROWS, :IN_F])

        m = {
            "x_pad": x_pad,
            "x_own": x_own,
            "idx16": np.ascontiguousarray(idx_pk),
            "dstloc": dstloc.astype(ml_dtypes.bfloat16),
            "deginv": dinv_col,
            "iota": iota_bf,
            "ones_row": ones_row,
            "ident": ident_bf,
            "ln_g_rep": ln_g_rep,
            "ln_b_rep": ln_b_rep,
        }
        m.update(wcast)
        m.update(brow)
        in_maps.append(m)
    return in_maps, ln_identity


def build_program(cfg, ln_identity):
    B, ROWS, GROWS = cfg.B, cfg.ROWS, cfg.GROWS
    CHUNK = cfg.CHUNK
    tiles_bc, tile_start, stream_base = cfg.tiles_bc, cfg.tile_start, cfg.stream_base
    CT, T_b, dcol_start = cfg.CT, cfg.T_b, cfg.dcol_start
    TOT_SLOTS = cfg.TOT_TILES * 128
    nc = bacc.Bacc("TRN2", target_bir_lowering=False, debug=False,
                   num_devices=NCORES, num_swdge_queues=4)

    x_pad = nc.dram_tensor("x_pad", [GROWS, GF], BF16, kind="ExternalInput")
    x_own = nc.dram_tensor("x_own", [ROWS, IN_F], BF16, kind="ExternalInput")
    idx_d = nc.dram_tensor("idx16", [128, TOT_SLOTS // 16], mybir.dt.int16,
                           kind="ExternalInput")
    dstloc_d = nc.dram_tensor("dstloc", [128, cfg.TOT_TILES], BF16,
                              kind="ExternalInput")
    deginv_d = nc.dram_tensor("deginv", [128, B], F32, kind="ExternalInput")
    iota_d = nc.dram_tensor("iota", [128, 128], BF16, kind="ExternalInput")
    ones_d = nc.dram_tensor("ones_row", [1, 128], BF16, kind="ExternalInput")
    ident_d = nc.dram_tensor("ident", [128, 128], BF16, kind="ExternalInput")
    lng_d = nc.dram_tensor("ln_g_rep", [128, HID], F32, kind="ExternalInput")
    lnb_d = nc.dram_tensor("ln_b_rep", [128, HID], F32, kind="ExternalInput")
    wd = {}
    for k, shp in [("Wl1", [IN_F, HID]), ("Wr1", [IN_F, HID]), ("Wres", [IN_F, HID]),
                   ("Wl2", [HID, HID]), ("Wr2", [HID, HID]),
                   ("Wl3", [HID, HID]), ("Wr3", [HID, HID]),
                   ("Wl4", [HID, 1]), ("Wr4", [HID, 1])]:
        wd[k] = nc.dram_tensor(k, shp, BF16, kind="ExternalInput")
    bd = {}
    for k in ["b1", "bres", "b2", "b3"]:
        bd[k] = nc.dram_tensor(k, [1, HID], BF16, kind="ExternalInput")
    bd["b4"] = nc.dram_tensor("b4", [1, 1], BF16, kind="ExternalInput")

    out_d = nc.dram_tensor("out", [ROWS], F32, kind="ExternalOutput")
    rg = [list(range(NCORES))]

    with tile.TileContext(nc) as tc:
        with (
            tc.tile_pool(name="dramp", bufs=1, space="DRAM") as dramp,
            tc.tile_pool(name="const", bufs=1) as constp,
            tc.tile_pool(name="meta", bufs=1) as metap,
            tc.tile_pool(name="gpool", bufs=10) as gpool,
            tc.tile_pool(name="ohpool", bufs=6) as ohpool,
            tc.tile_pool(name="spool", bufs=4) as spool,
            tc.tile_pool(name="hpool", bufs=4) as hpool,
            tc.tile_pool(name="outp", bufs=1) as outp,
            tc.tile_pool(name="ps", bufs=2, space="PSUM") as ps,
        ):
            h_own = [dramp.tile([ROWS, HID], BF16, tag=f"h_own{l}",
                                name=f"h_own{l}") for l in range(3)]
            h_full = [dramp.tile([GROWS, GF], BF16, tag=f"h_full{l}",
                                 name=f"h_full{l}", addr_space="Shared")
                      for l in range(3)]

            idx_t = metap.tile([128, TOT_SLOTS // 16], mybir.dt.int16)
            nc.sync.dma_start(out=idx_t[:], in_=idx_d[:])
            dstloc_t = metap.tile([128, cfg.TOT_TILES], BF16)
            nc.sync.dma_start(out=dstloc_t[:], in_=dstloc_d[:])
            deginv_t = metap.tile([128, B], F32)
            nc.sync.dma_start(out=deginv_t[:], in_=deginv_d[:])
            iota_t = constp.tile([128, 128], BF16)
            nc.sync.dma_start(out=iota_t[:], in_=iota_d[:])
            ones_t = constp.tile([1, 128], BF16)
            nc.sync.dma_start(out=ones_t[:], in_=ones_d[:])
            ident_t = constp.tile([128, 128], BF16)
            nc.sync.dma_start(out=ident_t[:], in_=ident_d[:])
            eps_t = constp.tile([128, 1], F32)
            nc.vector.memset(eps_t[:], 1e-5)
            lng_t = constp.tile([128, HID], F32)
            nc.sync.dma_start(out=lng_t[:], in_=lng_d[:])
            lnb_t = constp.tile([128, HID], F32)
            nc.sync.dma_start(out=lnb_t[:], in_=lnb_d[:])
            w_t = {}
            for k, h in wd.items():
                w_t[k] = constp.tile(list(h.shape), BF16, tag=f"w_{k}", name=f"w_{k}")
                nc.sync.dma_start(out=w_t[k][:], in_=h[:])
            b_t = {}
            for k, h in bd.items():
                b_t[k] = constp.tile(list(h.shape), BF16, tag=f"b_{k}", name=f"b_{k}")
                nc.sync.dma_start(out=b_t[k][:], in_=h[:])

            out_sb = outp.tile([128, B], F32)

            def build_onehot(b):
                tb = int(T_b[b])
                oh = ohpool.tile([128, cfg.Tmax * 128], BF16, tag="oh")
                nc.vector.tensor_tensor(
                    out=oh[:, :tb * 128].rearrange("p (t j) -> p t j", t=tb),
                    in0=iota_t[:, None, :].to_broadcast([128, tb, 128]),
                    in1=dstloc_t[:, dcol_start[b]:dcol_start[b] + tb
                                 ].to_broadcast([128, tb, 128]),
                    op=ALU.is_equal,
                )
                return oh

            NCALLS = [(int(CT[ch]) + WT - 1) // WT for ch in range(NCHUNK)]
            state = {}

            def new_layer(src_dram):
                state.clear()
                state.update(src=src_dram, G={}, nxt=[0] * NCHUNK)

            def issue_call(ch, k):
                lo = k * WT
                hi = min(int(CT[ch]), lo + WT)
                nt = hi - lo
                rows = nt * 128
                G = gpool.tile([128, WT * GF], BF16, tag="G", name=f"G_{ch}_{k}")
                base16 = (int(stream_base[ch]) + lo) * 8
                nc.gpsimd.dma_gather(
                    out_ap=G[:, :nt * GF].rearrange("p (t e) -> p t e", e=GF),
                    in_ap=state["src"][ch * CHUNK:(ch + 1) * CHUNK, :],
                    idxs_ap=idx_t[:, base16:base16 + rows // 16],
                    num_idxs=rows,
                    num_idxs_reg=rows,
                    elem_size=GF,
                    queue_num=ch,
                )
                state["G"][(ch, k)] = G

            def scatter(b, agg_psum, featL):
                for ch in range(NCHUNK):
                    need_hi = int(tile_start[b, ch] + tiles_bc[b, ch])
                    while state["nxt"][ch] * WT < need_hi:
                        issue_call(ch, state["nxt"][ch])
                        state["nxt"][ch] += 1
                oh = build_onehot(b)
                tb = int(T_b[b])
                j = 0
                for ch in range(NCHUNK):
                    for t in range(int(tiles_bc[b, ch])):
                        pos = int(tile_start[b, ch]) + t
                        G = state["G"][(ch, pos // WT)]
                        off = (pos % WT) * GF
                        nc.tensor.matmul(
                            agg_psum[:], lhsT=G[:, off:off + featL],
                            rhs=oh[:, j * 128:(j + 1) * 128],
                            start=(j == 0), stop=(j == tb - 1))
                        j += 1

            # =================== Layer 1 ===================
            new_layer(x_pad)
            for b in range(B):
                xblk = spool.tile([128, IN_F], BF16, tag="xblk")
                nc.sync.dma_start(out=xblk[:], in_=x_own[b * 128:(b + 1) * 128, :])
                xT_ps = ps.tile([IN_F, 128], BF16, tag="xT_ps", bufs=1)
                nc.tensor.transpose(xT_ps[:], xblk[:], ident_t[:])
                xT = spool.tile([IN_F, 128], BF16, tag="xT")
                nc.scalar.activation(xT[:], xT_ps[:], AF.Copy)

                agg_ps = ps.tile([IN_F, 128], F32, tag="agg", bufs=2)
                scatter(b, agg_ps, IN_F)
                aggT = spool.tile([IN_F, 128], BF16, tag="aggT1")
                nc.scalar.activation(aggT[:], agg_ps[:], AF.Copy)

                zA = ps.tile([128, HID], F32, tag="zA", bufs=2)
                nc.tensor.matmul(zA[:], lhsT=aggT[:], rhs=w_t["Wl1"][:],
                                 start=True, stop=True)
                zB = ps.tile([128, HID], F32, tag="zB", bufs=2)
                nc.tensor.matmul(zB[:], lhsT=xT[:], rhs=w_t["Wr1"][:],
                                 start=True, stop=False)
                nc.tensor.matmul(zB[:], lhsT=ones_t[:], rhs=b_t["b1"][:],
                                 start=False, stop=True)
                res = ps.tile([128, HID], F32, tag="res", bufs=1)
                nc.tensor.matmul(res[:], lhsT=xT[:], rhs=w_t["Wres"][:],
                                 start=True, stop=False)
                nc.tensor.matmul(res[:], lhsT=ones_t[:], rhs=b_t["bres"][:],
                                 start=False, stop=True)

                sA = spool.tile([128, HID], F32, tag="sA")
                nc.scalar.activation(sA[:], zA[:], AF.Copy,
                                     scale=deginv_t[:, b:b + 1])
                z = spool.tile([128, HID], F32, tag="z")
                nc.vector.tensor_tensor(out=z[:], in0=sA[:], in1=zB[:], op=ALU.add)

                # LayerNorm: reductions on DVE, pointwise chain on Scalar
                mu = spool.tile([128, 1], F32, tag="mu")
                nc.vector.reduce_sum(out=mu[:], in_=z[:], axis=mybir.AxisListType.X)
                negmu = spool.tile([128, 1], F32, tag="negmu")
                nc.scalar.activation(negmu[:], mu[:], AF.Copy, scale=-1.0 / HID)
                sq = spool.tile([128, HID], F32, tag="sq")
                nc.scalar.activation(sq[:], z[:], AF.Square, bias=negmu[:])
                var = spool.tile([128, 1], F32, tag="var")
                nc.vector.reduce_sum(out=var[:], in_=sq[:], axis=mybir.AxisListType.X)
                std = spool.tile([128, 1], F32, tag="std")
                nc.scalar.activation(std[:], var[:], AF.Sqrt, scale=1.0 / HID,
                                     bias=eps_t[:])
                rstd = spool.tile([128, 1], F32, tag="rstd")
                nc.vector.reciprocal(rstd[:], std[:])
                nmr = spool.tile([128, 1], F32, tag="nmr")
                nc.scalar.activation(nmr[:], negmu[:], AF.Copy, scale=rstd[:])

                if ln_identity:
                    zr = spool.tile([128, HID], F32, tag="zr")
                    nc.scalar.activation(zr[:], z[:], AF.Relu, scale=rstd[:],
                                         bias=nmr[:])
                else:
                    xc = spool.tile([128, HID], F32, tag="xc")
                    nc.vector.tensor_scalar(out=xc[:], in0=z[:], scalar1=negmu[:],
                                            scalar2=None, op0=ALU.add)
                    zn = spool.tile([128, HID], F32, tag="zn")
                    nc.scalar.activation(zn[:], xc[:], AF.Copy, scale=rstd[:])
                    nc.vector.tensor_tensor(out=zn[:], in0=zn[:], in1=lng_t[:],
                                            op=ALU.mult)
                    nc.vector.tensor_tensor(out=zn[:], in0=zn[:], in1=lnb_t[:],
                                            op=ALU.add)
                    zr = spool.tile([128, HID], F32, tag="zr")
                    nc.vector.tensor_scalar(out=zr[:], in0=zn[:], scalar1=0.0,
                                            scalar2=None, op0=ALU.max)

                h1 = hpool.tile([128, HID], BF16, tag="hsb")
                nc.vector.tensor_tensor(out=h1[:], in0=zr[:], in1=res[:], op=ALU.add)
                nc.sync.dma_start(out=h_own[0][b * 128:(b + 1) * 128, :], in_=h1[:])

            nc.gpsimd.collective_compute(
                "AllGather", ALU.bypass, replica_groups=rg,
                ins=[h_own[0][:]], outs=[h_full[0][:]])

            # =================== Layers 2,3 ===================
            for li, (wl, wr, bb) in enumerate([("Wl2", "Wr2", "b2"),
                                               ("Wl3", "Wr3", "b3")]):
                new_layer(h_full[li])
                for b in range(B):
                    hblk = spool.tile([128, HID], BF16, tag="hblk")
                    nc.sync.dma_start(
                        out=hblk[:], in_=h_own[li][b * 128:(b + 1) * 128, :])
                    hT_ps = ps.tile([HID, 128], BF16, tag="xT_ps", bufs=1)
                    nc.tensor.transpose(hT_ps[:], hblk[:], ident_t[:])
                    hT = spool.tile([HID, 128], BF16, tag="hT")
                    nc.scalar.activation(hT[:], hT_ps[:], AF.Copy)
                    agg_ps = ps.tile([HID, 128], F32, tag="agg", bufs=2)
                    scatter(b, agg_ps, HID)
                    aggT = spool.tile([HID, 128], BF16, tag="aggT2")
                    nc.scalar.activation(aggT[:], agg_ps[:], AF.Copy)

                    zA = ps.tile([128, HID], F32, tag="zA", bufs=2)
                    nc.tensor.matmul(zA[:], lhsT=aggT[:], rhs=w_t[wl][:],
                                     start=True, stop=True)
                    zB = ps.tile([128, HID], F32, tag="zB", bufs=2)
                    nc.tensor.matmul(zB[:], lhsT=hT[:], rhs=w_t[wr][:],
                                     start=True, stop=False)
                    nc.tensor.matmul(zB[:], lhsT=ones_t[:], rhs=b_t[bb][:],
                                     start=False, stop=True)

                    sA = spool.tile([128, HID], F32, tag="sA")
                    nc.scalar.activation(sA[:], zA[:], AF.Copy,
                                         scale=deginv_t[:, b:b + 1])
                    z = spool.tile([128, HID], F32, tag="z")
                    nc.vector.tensor_tensor(out=z[:], in0=sA[:], in1=zB[:],
                                            op=ALU.add)
                    h2 = hpool.tile([128, HID], BF16, tag="hsb")
                    nc.scalar.activation(h2[:], z[:], AF.Relu)
                    nc.sync.dma_start(
                        out=h_own[li + 1][b * 128:(b + 1) * 128, :], in_=h2[:])

                nc.gpsimd.collective_compute(
                    "AllGather", ALU.bypass, replica_groups=rg,
                    ins=[h_own[li + 1][:]], outs=[h_full[li + 1][:]])

            # =================== Layer 4 ===================
            new_layer(h_full[2])
            for b in range(B):
                hblk = spool.tile([128, HID], BF16, tag="hblk")
                nc.sync.dma_start(
                    out=hblk[:], in_=h_own[2][b * 128:(b + 1) * 128, :])
                hT_ps = ps.tile([HID, 128], BF16, tag="xT_ps", bufs=1)
                nc.tensor.transpose(hT_ps[:], hblk[:], ident_t[:])
                hT = spool.tile([HID, 128], BF16, tag="hT")
                nc.scalar.activation(hT[:], hT_ps[:], AF.Copy)
                agg_ps = ps.tile([HID, 128], F32, tag="agg", bufs=2)
                scatter(b, agg_ps, HID)
                aggT = spool.tile([HID, 128], BF16, tag="aggT2")
                nc.scalar.activation(aggT[:], agg_ps[:], AF.Copy)

                oA = ps.tile([128, 1], F32, tag="zA", bufs=2)
                nc.tensor.matmul(oA[:], lhsT=aggT[:], rhs=w_t["Wl4"][:],
                                 start=True, stop=True)
                oB = ps.tile([128, 1], F32, tag="zB", bufs=2)
                nc.tensor.matmul(oB[:], lhsT=hT[:], rhs=w_t["Wr4"][:],
                                 start=True, stop=False)
                nc.tensor.matmul(oB[:], lhsT=ones_t[:], rhs=b_t["b4"][:],
                                 start=False, stop=True)
                t4 = spool.tile([128, 1], F32, tag="t4")
                nc.scalar.activation(t4[:], oA[:], AF.Copy,
                                     scale=deginv_t[:, b:b + 1])
                nc.vector.tensor_tensor(out=out_sb[:, b:b + 1], in0=t4[:],
                                        in1=oB[:], op=ALU.add)

            nc.sync.dma_start(
                out=out_d[:].rearrange("(b p) -> p b", p=128), in_=out_sb[:])

    nc.compile()
    return nc


# ---------------------------------------------------------------------------
# Self-contained entry point


def _ensure_ntff_hook_package():
    """Best-effort: make antenv.axon_hooks importable for future interpreters
    so trn_boot can register the NTFF profiling hook. Harmless if present."""
    import os
    site = "/root/.axon_site"
    try:
        pkg = os.path.join(site, "antenv")
        os.makedirs(pkg, exist_ok=True)
        init = os.path.join(pkg, "__init__.py")
        if not os.path.exists(init):
            with open(init, "w") as f:
                f.write("import pkgutil\n__path__ = pkgutil.extend_path(__path__, __name__)\n")
        hooks = os.path.join(pkg, "axon_hooks.py")
        if not os.path.exists(hooks):
            with open(hooks, "w") as f:
                f.write(
                    "_H = None\n"
                    "def set_axon_ntff_profile_hook(h):\n"
                    "    global _H\n"
                    "    _H = h\n"
                    "def get_axon_ntff_profile_hook():\n"
                    "    return _H\n")
    except Exception:
        pass


_ensure_ntff_hook_package()

_CACHE = {}
LAST_EXEC_NS = None


def kernel(**inputs):
    global LAST_EXEC_NS
    x = np.asarray(inputs["x"], np.float32)
    edge_index = np.asarray(inputs["edge_index"])
    cfg = Cfg(x.shape[0])
    weights = {k: v for k, v in inputs.items() if k not in ("x", "edge_index")}
    in_maps, ln_identity = preprocess(cfg, x, edge_index, weights)

    key = (x.shape, edge_index.shape, tuple(cfg.T_b.tolist()), ln_identity)
    if key in _CACHE:
        nc = _CACHE[key]
    else:
        nc = build_program(cfg, ln_identity)
        _CACHE[key] = nc

    from concourse.bass_utils import run_bass_kernel_spmd
    import concourse.bass_utils as bu
    bu.upload_artifacts = lambda d: d

    res = None
    try:
        res = run_bass_kernel_spmd(nc, in_maps, core_ids=list(range(NCORES)),
                                   trace=True)
        LAST_EXEC_NS = res.exec_time_ns
    except Exception:
        res = None
    if res is None:
        res = run_bass_kernel_spmd(nc, in_maps, core_ids=list(range(NCORES)),
                                   trace=False)
        LAST_EXEC_NS = None
    outs = [res.results[c]["out"] for c in range(NCORES)]
    return np.concatenate([np.asarray(o)[:cfg.NPC] for o in outs]).astype(np.float32)
